# revision 38
# baseline (speedup 1.0000x reference)
"""Causal self-attention with RoPE on 8 Trainium2 NeuronCores.

Sharding: core c handles batch b = c//2 and head-group g = c%2 (8 of the 16
heads).  Wq/Wk/Wv are column-sharded (per head group), Wp is row-sharded;
each core computes a partial output projection for its batch and the host
sums the two partials per batch (the row-parallel unshard).

Device layouts (per core):
  xT    [C=1024, T=2048]  x transposed (contraction-friendly)
  wqT/wkT/wvT [1024, 512] W shard transposed ([c, d_local])
  wpT   [512, 1024]       Wp shard transposed ([c_local, e])
  cosT/sinT [128, 2048]   RoPE tables in [d, t] layout (2 head replicas,
                          sign folded into sinT for the rotate-half term)
  out   [2048, 1024] f32  partial projection output

Inside: q^T,k^T computed in [d, t] layout, v in [t, d]; scores computed
transposed (S^T = [k, t_q]) so softmax-normalizer and attention-output both
come from plain matmuls (V gets an appended ones-column to produce the
softmax denominator for free); causal mask applied post-exp via
affine_select (exact zeros).  All matmuls bf16 with fp32 PSUM accumulate.
"""

import sys

sys.path.insert(0, "/opt/trn_rl_repo")

import ml_dtypes
import numpy as np

import concourse.bass as bass
import concourse.mybir as mybir
import concourse.tile as tile
from concourse import bacc
from concourse.bass_utils import run_bass_kernel_spmd

BF = mybir.dt.bfloat16
F32 = mybir.dt.float32
NPBF = ml_dtypes.bfloat16

B, T, C = 4, 2048, 1024
H, D = 16, 64
HL, DL = 8, 512  # heads / channels per core
NCT = C // 128  # 8 contraction tiles
NTT = T // 512  # 4 big time windows
NT16 = T // 128  # 16 small time windows
ROPE_BASE = 10000.0

SWAP_MASK = [i ^ 1 for i in range(32)]


def _build_nc():
    nc = bacc.Bacc("TRN2", target_bir_lowering=False, debug=False)

    xT_d = nc.dram_tensor("xT", [C, T], BF, kind="ExternalInput")
    wq_d = nc.dram_tensor("wqT", [C, DL], BF, kind="ExternalInput")
    wk_d = nc.dram_tensor("wkT", [C, DL], BF, kind="ExternalInput")
    wv_d = nc.dram_tensor("wvT", [C, DL], BF, kind="ExternalInput")
    wp_d = nc.dram_tensor("wpT", [DL, C], BF, kind="ExternalInput")
    cos_d = nc.dram_tensor("cosT", [128, T], BF, kind="ExternalInput")
    sin_d = nc.dram_tensor("sinT", [128, T], BF, kind="ExternalInput")
    out_d = nc.dram_tensor("out", [T, C], F32, kind="ExternalOutput")

    with tile.TileContext(nc) as tc:
        _body(nc, tc, xT_d, wq_d, wk_d, wv_d, wp_d, cos_d, sin_d, out_d)
    nc.compile()
    return nc


def _body(nc, tc, xT_d, wq_d, wk_d, wv_d, wp_d, cos_d, sin_d, out_d):
    import contextlib

    ctx = contextlib.ExitStack()
    with ctx:
        const = ctx.enter_context(tc.tile_pool(name="const", bufs=1))
        work = ctx.enter_context(tc.tile_pool(name="work", bufs=2))
        psum = ctx.enter_context(tc.tile_pool(name="psum", bufs=1, space="PSUM"))

        # ---- resident SBUF tensors -------------------------------------
        x_sb = const.tile([128, NCT, T], BF)
        nc.sync.dma_start(out=x_sb, in_=xT_d[:].rearrange("(a p) t -> p a t", p=128))
        wq_sb = const.tile([128, NCT, DL], BF)
        nc.sync.dma_start(out=wq_sb, in_=wq_d[:].rearrange("(a p) d -> p a d", p=128))
        wk_sb = const.tile([128, NCT, DL], BF)
        nc.sync.dma_start(out=wk_sb, in_=wk_d[:].rearrange("(a p) d -> p a d", p=128))
        wv_sb = const.tile([128, NCT, DL], BF)
        nc.sync.dma_start(out=wv_sb, in_=wv_d[:].rearrange("(a p) d -> p a d", p=128))
        wp_sb = const.tile([128, 4, C], BF)
        nc.sync.dma_start(out=wp_sb, in_=wp_d[:].rearrange("(a p) e -> p a e", p=128))
        cos_sb = const.tile([128, T], BF)
        nc.sync.dma_start(out=cos_sb, in_=cos_d[:])
        sin_sb = const.tile([128, T], BF)
        nc.sync.dma_start(out=sin_sb, in_=sin_d[:])

        # v in [t, h, d(+ones)] layout; col 64 of each head group is 1.0
        v_sb = const.tile([128, NT16, HL, 65], BF)
        nc.vector.memset(v_sb[:, :, :, 64], 1.0)

        qr_sb = const.tile([128, 4, T], BF)  # q^T after rope, 4 head-pair tiles
        kr_sb = const.tile([128, 4, T], BF)
        yT_sb = const.tile([128, 4, T], BF)  # attention out, pre-projection

        # ---- per-window phase bodies -----------------------------------
        def rope_evac(ps, tsl, nm):
            ev = work.tile([128, 512], BF, tag="ev", bufs=3, name=f"ev{nm}")
            nc.vector.tensor_copy(ev, ps)
            sh = work.tile([128, 512], BF, tag="sh", bufs=3, name=f"sh{nm}")
            nc.vector.stream_shuffle(sh, ev, SWAP_MASK)
            t1 = work.tile([128, 512], BF, tag="t1", bufs=3, name=f"t1{nm}")
            nc.vector.tensor_mul(t1, sh, sin_sb[:, tsl])
            t2 = work.tile([128, 512], BF, tag="t2", bufs=3, name=f"t2{nm}")
            nc.vector.tensor_mul(t2, ev, cos_sb[:, tsl])
            return t1, t2

        def proj_qkv_piece(tt, m):
            """Generator: q^T,k^T (+rope) for pair m and v for t16=4tt+m of
            time window tt.  Yields between matmuls so the caller can
            interleave these as PE filler inside attention."""
            tsl = slice(tt * 512, (tt + 1) * 512)
            dsl = slice(m * 128, (m + 1) * 128)
            for W, dst, nm in ((wq_sb, qr_sb, "q"), (wk_sb, kr_sb, "k")):
                ps = psum.tile([128, 512], F32, tag="pj", bufs=2, name=f"{nm}p{tt}_{m}")
                for ct in range(NCT):
                    nc.tensor.matmul(
                        ps,
                        lhsT=W[:, ct, dsl],
                        rhs=x_sb[:, ct, tsl],
                        start=(ct == 0),
                        stop=(ct == NCT - 1),
                    )
                    yield
                t1, t2 = rope_evac(ps, tsl, f"{nm}{tt}_{m}")
                nc.vector.tensor_add(dst[:, m, tsl], t1, t2)
                yield
            t16 = 4 * tt + m
            vp = psum.tile([128, 512], F32, tag="pj", bufs=2, name=f"vp{t16}")
            for ct in range(NCT):
                nc.tensor.matmul(
                    vp,
                    lhsT=x_sb[:, ct, t16 * 128 : (t16 + 1) * 128],
                    rhs=wv_sb[:, ct, :],
                    start=(ct == 0),
                    stop=(ct == NCT - 1),
                )
                yield
            nc.scalar.copy(
                v_sb[:, t16, :, 0:64], vp.rearrange("p (h d) -> p h d", h=HL)
            )
            yield

        def attention(m, qt, filler=None):
            """Both heads of pair m, query window qt (row-packed on PE).

            All S^T matmuls are issued first (exps stream behind on ACT),
            with `filler` (a generator of independent PE work) interleaved;
            the AV matmuls run last, by which time every exp has finished —
            the PE never waits on the scalar engine."""
            qsl = slice(qt * 512, (qt + 1) * 512)
            nk = 4 * qt + 4
            yxs = [
                psum.tile([65, 512], F32, tag="yx", bufs=2, name=f"yx{m}_{qt}_{h2}")
                for h2 in (0, 1)
            ]
            def emit_st(ki):
                # one [128,1024] tile: head A scores in cols 0-511 (bank 1),
                # head B in cols 512-1023 (bank 2); the two matmuls run
                # concurrently in disjoint PE row groups (K=64 each).
                # Diagonal k-tiles only compute the live (unmasked) q-range
                # [q0, 512) — q columns below 128*(ki-4qt) are fully masked.
                q0 = max(0, 128 * ki - 512 * qt)
                w = 512 - q0
                st = psum.tile([128, 1024], F32, tag="st", bufs=2, name=f"st{m}_{qt}_{ki}")
                for h2 in (0, 1):
                    rsl = slice(64 * h2, 64 * h2 + 64)
                    nc.tensor.matmul(
                        st[:, h2 * 512 + q0 : (h2 + 1) * 512],
                        lhsT=kr_sb[rsl, m, ki * 128 : (ki + 1) * 128],
                        rhs=qr_sb[rsl, m, qt * 512 + q0 : (qt + 1) * 512],
                        start=True,
                        stop=True,
                    )
                pt = work.tile([128, 1024], BF, tag="pt", bufs=10, name=f"pt{m}_{qt}_{ki}")
                stv = st.rearrange("p (g c) -> p g c", g=2)[:, :, q0:512]
                ptv = pt.rearrange("p (g c) -> p g c", g=2)[:, :, q0:512]
                nc.scalar.activation(
                    ptv, stv, mybir.ActivationFunctionType.Exp, scale=0.125
                )
                if ki >= 4 * qt:  # diagonal block: causal mask (both halves)
                    nc.gpsimd.affine_select(
                        ptv,
                        ptv,
                        pattern=[[0, 2], [1, w]],
                        compare_op=mybir.AluOpType.is_ge,
                        fill=0.0,
                        base=0,
                        channel_multiplier=-1,
                    )
                return pt

            def emit_av(ki, pt):
                q0 = max(0, 128 * ki - 512 * qt)
                for h2 in (0, 1):
                    nc.tensor.matmul(
                        yxs[h2][:, q0:512],
                        lhsT=v_sb[:, ki, 2 * m + h2, :],
                        rhs=pt[:, h2 * 512 + q0 : (h2 + 1) * 512],
                        start=(ki == 0),
                        stop=(ki == nk - 1),
                    )

            # blocks of 8 k-tiles bound the number of live pt tiles
            for k0 in range(0, nk, 8):
                kis = list(range(k0, min(k0 + 8, nk)))
                pts = []
                for ki in kis:
                    pts.append(emit_st(ki))
                    if filler is not None:  # ~2 independent PE ops per k-tile
                        for _ in range(2):
                            next(filler, None)
                for ki, pt in zip(kis, pts):
                    emit_av(ki, pt)

            # evacuate PSUM immediately for BOTH heads (frees the yx banks
            # fast); the slow division chain is deferred to the end of the
            # tt iteration — its result is first needed a full window later.
            ysrs = []
            for h2 in (0, 1):
                ysr = work.tile(
                    [65, 512], F32, tag="ysr", bufs=8, name=f"ysr{m}_{qt}_{h2}"
                )
                nc.vector.tensor_copy(ysr, yxs[h2])
                ysrs.append(ysr)

            def epilogue():
                rls = []
                for h2 in (0, 1):
                    rl = work.tile(
                        [1, 512], F32, tag="rl", bufs=2, name=f"rl{m}_{qt}_{h2}"
                    )
                    nc.vector.reciprocal(rl, ysrs[h2][64:65, :])
                    rls.append(rl)
                for h2 in (0, 1):
                    rsl = slice(64 * h2, 64 * h2 + 64)
                    rlb = work.tile(
                        [64, 512], F32, tag="rlb", bufs=2, name=f"rlb{m}_{qt}_{h2}"
                    )
                    nc.gpsimd.partition_broadcast(rlb, rls[h2])
                    nc.vector.tensor_mul(yT_sb[rsl, m, qsl], ysrs[h2][0:64, :], rlb)

            return epilogue

        def proj_out_piece(t16):
            """Generator: output-projection partial for time window t16."""
            t16sl = slice(t16 * 128, (t16 + 1) * 128)
            osb = work.tile([128, C], F32, tag="osb", bufs=3, name=f"osb{t16}")
            for e2 in (0, 1):
                op = psum.tile([128, 512], F32, tag="pj", bufs=2, name=f"op{t16}_{e2}")
                for hdt in range(4):
                    nc.tensor.matmul(
                        op,
                        lhsT=yT_sb[:, hdt, t16sl],
                        rhs=wp_sb[:, hdt, e2 * 512 : (e2 + 1) * 512],
                        start=(hdt == 0),
                        stop=(hdt == 3),
                    )
                    yield
                nc.scalar.copy(osb[:, e2 * 512 : (e2 + 1) * 512], op)
                yield
            nc.sync.dma_start(out=out_d[t16sl, :], in_=osb)
            yield

        def chain(*gens):
            for g in gens:
                yield from g

        def drain(g):
            for _ in g:
                pass

        # ---- merged schedule ------------------------------------------
        # attention at query-window qt needs q/k/v only through window qt.
        # Window 0's projections run up front; afterwards, window tt+1's
        # projections and window tt-1's output projection are fed as PE
        # filler INSIDE window tt's attention pairs, so the PE always has
        # independent matmuls while ACT streams the softmax exps.
        for m in range(4):
            drain(proj_qkv_piece(0, m))
        for tt in range(NTT):
            epilogues = []
            for m in range(4):
                gens = []
                if tt + 1 < NTT:
                    gens.append(proj_qkv_piece(tt + 1, m))
                if tt > 0:
                    gens.append(proj_out_piece(4 * (tt - 1) + m))
                filler = chain(*gens)
                ep = attention(m, tt, filler)
                drain(filler)
                # stagger division chains one pair behind so the slow DVE
                # recips never sit ahead of latency-critical queue entries
                if epilogues:
                    epilogues.pop(0)()
                epilogues.append(ep)
            for ep in epilogues:
                ep()
        for m in range(4):
            drain(proj_out_piece(12 + m))


_NC_CACHE = None
LAST_RESULT = None


def _get_nc():
    global _NC_CACHE
    if _NC_CACHE is None:
        _NC_CACHE = _build_nc()
    return _NC_CACHE


def _rope_tables(start_pos):
    inv = 1.0 / (ROPE_BASE ** (np.arange(0, D, 2, dtype=np.float32) / D))
    t = np.arange(T, dtype=np.float32) + np.float32(start_pos)
    freqs = t[:, None] * inv[None, :]  # [T, 32]
    emb = np.concatenate([freqs, freqs], axis=-1)  # [T, 64]
    cos = np.cos(emb).T  # [64, T]
    sin = np.sin(emb).T
    sgn = np.where(np.arange(D) % 2 == 0, -1.0, 1.0).astype(np.float32)
    cosT = np.tile(cos, (2, 1))
    sinT = np.tile(sin * sgn[:, None], (2, 1))
    return cosT.astype(NPBF), sinT.astype(NPBF)


def kernel(x, Wq, Wk, Wv, Wp, start_pos):
    x = np.asarray(x, dtype=np.float32)
    Wq = np.asarray(Wq, dtype=np.float32)
    Wk = np.asarray(Wk, dtype=np.float32)
    Wv = np.asarray(Wv, dtype=np.float32)
    Wp = np.asarray(Wp, dtype=np.float32)
    cosT, sinT = _rope_tables(int(start_pos))

    nc = _get_nc()
    in_maps = []
    for c in range(8):
        b, g = divmod(c, 2)
        hs = slice(g * DL, (g + 1) * DL)
        in_maps.append(
            {
                "xT": np.ascontiguousarray(x[b].T).astype(NPBF),
                "wqT": np.ascontiguousarray(Wq[hs, :].T).astype(NPBF),
                "wkT": np.ascontiguousarray(Wk[hs, :].T).astype(NPBF),
                "wvT": np.ascontiguousarray(Wv[hs, :].T).astype(NPBF),
                "wpT": np.ascontiguousarray(Wp[:, hs].T).astype(NPBF),
                "cosT": cosT,
                "sinT": sinT,
            }
        )
    res = run_bass_kernel_spmd(nc, in_maps, core_ids=list(range(8)))
    global LAST_RESULT
    LAST_RESULT = res
    outs = [r["out"] for r in res.results]
    full = np.stack(
        [outs[2 * b] + outs[2 * b + 1] for b in range(B)], axis=0
    )
    return full.astype(np.float32)


if __name__ == "__main__":
    nc = _get_nc()
    print("built ok:", len(nc.m.functions[0].blocks if hasattr(nc.m.functions[0], 'blocks') else []), "blocks")


# revision 41
# speedup vs baseline: 1.1488x; 1.1488x over previous
"""Causal self-attention with RoPE on 8 Trainium2 NeuronCores.

Sharding: core c handles batch b = c//2 and head-group g = c%2 (8 of the 16
heads).  Wq/Wk/Wv are column-sharded (per head group), Wp is row-sharded;
each core computes a partial output projection for its batch and the host
sums the two partials per batch (the row-parallel unshard).

Device layouts (per core):
  xT    [C=1024, T=2048]  x transposed (contraction-friendly)
  wqT/wkT/wvT [1024, 512] W shard transposed ([c, d_local])
  wpT   [512, 1024]       Wp shard transposed ([c_local, e])
  cosT/sinT [128, 2048]   RoPE tables in [d, t] layout (2 head replicas,
                          sign folded into sinT for the rotate-half term)
  out   [2048, 1024] f32  partial projection output

Inside: q^T,k^T computed in [d, t] layout, v in [t, d]; scores computed
transposed (S^T = [k, t_q]) so softmax-normalizer and attention-output both
come from plain matmuls (V gets an appended ones-column to produce the
softmax denominator for free); causal mask applied post-exp via
affine_select (exact zeros).  All matmuls bf16 with fp32 PSUM accumulate.
"""

import sys

sys.path.insert(0, "/opt/trn_rl_repo")

import ml_dtypes
import numpy as np

import concourse.bass as bass
import concourse.mybir as mybir
import concourse.tile as tile
from concourse import bacc
from concourse.bass_utils import run_bass_kernel_spmd

BF = mybir.dt.bfloat16
F32 = mybir.dt.float32
NPBF = ml_dtypes.bfloat16

B, T, C = 4, 2048, 1024
H, D = 16, 64
HL, DL = 8, 512  # heads / channels per core
NCT = C // 128  # 8 contraction tiles
NTT = T // 512  # 4 big time windows
NT16 = T // 128  # 16 small time windows
ROPE_BASE = 10000.0

SWAP_MASK = [i ^ 1 for i in range(32)]


def _build_nc():
    nc = bacc.Bacc("TRN2", target_bir_lowering=False, debug=False)

    xT_d = nc.dram_tensor("xT", [C, T], BF, kind="ExternalInput")
    wq_d = nc.dram_tensor("wqT", [C, DL], BF, kind="ExternalInput")
    wk_d = nc.dram_tensor("wkT", [C, DL], BF, kind="ExternalInput")
    wv_d = nc.dram_tensor("wvT", [C, DL], BF, kind="ExternalInput")
    wp_d = nc.dram_tensor("wpT", [DL, C], BF, kind="ExternalInput")
    cos_d = nc.dram_tensor("cosT", [128, T], BF, kind="ExternalInput")
    sin_d = nc.dram_tensor("sinT", [128, T], BF, kind="ExternalInput")
    out_d = nc.dram_tensor("out", [T, C], F32, kind="ExternalOutput")

    with tile.TileContext(nc) as tc:
        _body(nc, tc, xT_d, wq_d, wk_d, wv_d, wp_d, cos_d, sin_d, out_d)
    nc.compile()
    return nc


def _body(nc, tc, xT_d, wq_d, wk_d, wv_d, wp_d, cos_d, sin_d, out_d):
    import contextlib

    ctx = contextlib.ExitStack()
    with ctx:
        const = ctx.enter_context(tc.tile_pool(name="const", bufs=1))
        work = ctx.enter_context(tc.tile_pool(name="work", bufs=2))
        psum = ctx.enter_context(tc.tile_pool(name="psum", bufs=1, space="PSUM"))

        # ---- resident SBUF tensors -------------------------------------
        x_sb = const.tile([128, NCT, T], BF)
        nc.sync.dma_start(out=x_sb, in_=xT_d[:].rearrange("(a p) t -> p a t", p=128))
        wq_sb = const.tile([128, NCT, DL], BF)
        nc.sync.dma_start(out=wq_sb, in_=wq_d[:].rearrange("(a p) d -> p a d", p=128))
        wk_sb = const.tile([128, NCT, DL], BF)
        nc.sync.dma_start(out=wk_sb, in_=wk_d[:].rearrange("(a p) d -> p a d", p=128))
        wv_sb = const.tile([128, NCT, DL], BF)
        nc.sync.dma_start(out=wv_sb, in_=wv_d[:].rearrange("(a p) d -> p a d", p=128))
        wp_sb = const.tile([128, 4, C], BF)
        nc.sync.dma_start(out=wp_sb, in_=wp_d[:].rearrange("(a p) e -> p a e", p=128))
        cos_sb = const.tile([128, T], BF)
        nc.sync.dma_start(out=cos_sb, in_=cos_d[:])
        sin_sb = const.tile([128, T], BF)
        nc.sync.dma_start(out=sin_sb, in_=sin_d[:])

        # v in [t, h, d(+ones)] layout; col 64 of each head group is 1.0
        v_sb = const.tile([128, NT16, HL, 65], BF)
        nc.vector.memset(v_sb[:, :, :, 64], 1.0)

        qr_sb = const.tile([128, 4, T], BF)  # q^T after rope, 4 head-pair tiles
        kr_sb = const.tile([128, 4, T], BF)
        yT_sb = const.tile([128, 4, T], BF)  # attention out, pre-projection

        # ---- per-window phase bodies -----------------------------------
        def rope_evac(ps, tsl, nm):
            ev = work.tile([128, 512], BF, tag="ev", bufs=3, name=f"ev{nm}")
            nc.vector.tensor_copy(ev, ps)
            sh = work.tile([128, 512], BF, tag="sh", bufs=3, name=f"sh{nm}")
            nc.vector.stream_shuffle(sh, ev, SWAP_MASK)
            t1 = work.tile([128, 512], BF, tag="t1", bufs=3, name=f"t1{nm}")
            nc.vector.tensor_mul(t1, sh, sin_sb[:, tsl])
            t2 = work.tile([128, 512], BF, tag="t2", bufs=3, name=f"t2{nm}")
            nc.vector.tensor_mul(t2, ev, cos_sb[:, tsl])
            return t1, t2

        def proj_qkv_piece(tt, m):
            """Generator: q^T,k^T (+rope) for pair m and v for t16=4tt+m of
            time window tt.  Yields between matmuls so the caller can
            interleave these as PE filler inside attention."""
            tsl = slice(tt * 512, (tt + 1) * 512)
            dsl = slice(m * 128, (m + 1) * 128)
            for W, dst, nm in ((wq_sb, qr_sb, "q"), (wk_sb, kr_sb, "k")):
                ps = psum.tile([128, 512], F32, tag="pj", bufs=2, name=f"{nm}p{tt}_{m}")
                for ct in range(NCT):
                    nc.tensor.matmul(
                        ps,
                        lhsT=W[:, ct, dsl],
                        rhs=x_sb[:, ct, tsl],
                        start=(ct == 0),
                        stop=(ct == NCT - 1),
                    )
                    yield
                t1, t2 = rope_evac(ps, tsl, f"{nm}{tt}_{m}")
                nc.vector.tensor_add(dst[:, m, tsl], t1, t2)
                yield
            t16 = 4 * tt + m
            vp = psum.tile([128, 512], F32, tag="pj", bufs=2, name=f"vp{t16}")
            for ct in range(NCT):
                nc.tensor.matmul(
                    vp,
                    lhsT=x_sb[:, ct, t16 * 128 : (t16 + 1) * 128],
                    rhs=wv_sb[:, ct, :],
                    start=(ct == 0),
                    stop=(ct == NCT - 1),
                )
                yield
            nc.vector.tensor_copy(
                v_sb[:, t16, :, 0:64], vp.rearrange("p (h d) -> p h d", h=HL)
            )
            yield

        def attention(m, qt, filler=None):
            """Both heads of pair m, query window qt (row-packed on PE).

            All S^T matmuls are issued first (exps stream behind on ACT),
            with `filler` (a generator of independent PE work) interleaved;
            the AV matmuls run last, by which time every exp has finished —
            the PE never waits on the scalar engine."""
            qsl = slice(qt * 512, (qt + 1) * 512)
            nk = 4 * qt + 4
            yxs = [
                psum.tile([65, 512], F32, tag="yx", bufs=2, name=f"yx{m}_{qt}_{h2}")
                for h2 in (0, 1)
            ]
            def emit_st(ki):
                # one [128,1024] tile: head A scores in cols 0-511 (bank 1),
                # head B in cols 512-1023 (bank 2); the two matmuls run
                # concurrently in disjoint PE row groups (K=64 each).
                # Diagonal k-tiles only compute the live (unmasked) q-range
                # [q0, 512) — q columns below 128*(ki-4qt) are fully masked.
                q0 = max(0, 128 * ki - 512 * qt)
                w = 512 - q0
                st = psum.tile([128, 1024], F32, tag="st", bufs=2, name=f"st{m}_{qt}_{ki}")
                for h2 in (0, 1):
                    rsl = slice(64 * h2, 64 * h2 + 64)
                    nc.tensor.matmul(
                        st[:, h2 * 512 + q0 : (h2 + 1) * 512],
                        lhsT=kr_sb[rsl, m, ki * 128 : (ki + 1) * 128],
                        rhs=qr_sb[rsl, m, qt * 512 + q0 : (qt + 1) * 512],
                        start=True,
                        stop=True,
                    )
                pt = work.tile([128, 1024], BF, tag="pt", bufs=10, name=f"pt{m}_{qt}_{ki}")
                stv = st.rearrange("p (g c) -> p g c", g=2)[:, :, q0:512]
                ptv = pt.rearrange("p (g c) -> p g c", g=2)[:, :, q0:512]
                nc.scalar.activation(
                    ptv, stv, mybir.ActivationFunctionType.Exp, scale=0.125
                )
                if ki >= 4 * qt:  # diagonal block: causal mask (both halves)
                    nc.gpsimd.affine_select(
                        ptv,
                        ptv,
                        pattern=[[0, 2], [1, w]],
                        compare_op=mybir.AluOpType.is_ge,
                        fill=0.0,
                        base=0,
                        channel_multiplier=-1,
                    )
                return pt

            def emit_av(ki, pt):
                q0 = max(0, 128 * ki - 512 * qt)
                for h2 in (0, 1):
                    nc.tensor.matmul(
                        yxs[h2][:, q0:512],
                        lhsT=v_sb[:, ki, 2 * m + h2, :],
                        rhs=pt[:, h2 * 512 + q0 : (h2 + 1) * 512],
                        start=(ki == 0),
                        stop=(ki == nk - 1),
                    )

            # blocks of 8 k-tiles bound the number of live pt tiles
            for k0 in range(0, nk, 8):
                kis = list(range(k0, min(k0 + 8, nk)))
                pts = []
                for ki in kis:
                    pts.append(emit_st(ki))
                    if filler is not None:  # ~2 independent PE ops per k-tile
                        for _ in range(2):
                            next(filler, None)
                for ki, pt in zip(kis, pts):
                    emit_av(ki, pt)

            # evacuate PSUM immediately for BOTH heads (frees the yx banks
            # fast); the slow division chain is deferred to the end of the
            # tt iteration — its result is first needed a full window later.
            ysrs = []
            for h2 in (0, 1):
                ysr = work.tile(
                    [65, 512], F32, tag="ysr", bufs=8, name=f"ysr{m}_{qt}_{h2}"
                )
                nc.vector.tensor_copy(ysr, yxs[h2])
                ysrs.append(ysr)

            def epilogue():
                rls = []
                for h2 in (0, 1):
                    rl = work.tile(
                        [1, 512], F32, tag="rl", bufs=2, name=f"rl{m}_{qt}_{h2}"
                    )
                    nc.vector.reciprocal(rl, ysrs[h2][64:65, :])
                    rls.append(rl)
                for h2 in (0, 1):
                    rsl = slice(64 * h2, 64 * h2 + 64)
                    rlb = work.tile(
                        [64, 512], F32, tag="rlb", bufs=2, name=f"rlb{m}_{qt}_{h2}"
                    )
                    nc.gpsimd.partition_broadcast(rlb, rls[h2])
                    nc.vector.tensor_mul(yT_sb[rsl, m, qsl], ysrs[h2][0:64, :], rlb)

            return epilogue

        def proj_out_piece(t16):
            """Generator: output-projection partial for time window t16."""
            t16sl = slice(t16 * 128, (t16 + 1) * 128)
            osb = work.tile([128, C], F32, tag="osb", bufs=3, name=f"osb{t16}")
            for e2 in (0, 1):
                op = psum.tile([128, 512], F32, tag="pj", bufs=2, name=f"op{t16}_{e2}")
                for hdt in range(4):
                    nc.tensor.matmul(
                        op,
                        lhsT=yT_sb[:, hdt, t16sl],
                        rhs=wp_sb[:, hdt, e2 * 512 : (e2 + 1) * 512],
                        start=(hdt == 0),
                        stop=(hdt == 3),
                    )
                    yield
                nc.vector.tensor_copy(osb[:, e2 * 512 : (e2 + 1) * 512], op)
                yield
            nc.sync.dma_start(out=out_d[t16sl, :], in_=osb)
            yield

        def chain(*gens):
            for g in gens:
                yield from g

        def drain(g):
            for _ in g:
                pass

        # ---- merged schedule ------------------------------------------
        # attention at query-window qt needs q/k/v only through window qt.
        # Window 0's projections run up front; afterwards, window tt+1's
        # projections and window tt-1's output projection are fed as PE
        # filler INSIDE window tt's attention pairs, so the PE always has
        # independent matmuls while ACT streams the softmax exps.
        for m in range(4):
            drain(proj_qkv_piece(0, m))
        for tt in range(NTT):
            epilogues = []
            for m in range(4):
                gens = []
                if tt + 1 < NTT:
                    gens.append(proj_qkv_piece(tt + 1, m))
                if tt > 0:
                    gens.append(proj_out_piece(4 * (tt - 1) + m))
                filler = chain(*gens)
                epilogues.append(attention(m, tt, filler))
                drain(filler)
            for ep in epilogues:
                ep()
        for m in range(4):
            drain(proj_out_piece(12 + m))


_NC_CACHE = None
LAST_RESULT = None


def _get_nc():
    global _NC_CACHE
    if _NC_CACHE is None:
        _NC_CACHE = _build_nc()
    return _NC_CACHE


def _rope_tables(start_pos):
    inv = 1.0 / (ROPE_BASE ** (np.arange(0, D, 2, dtype=np.float32) / D))
    t = np.arange(T, dtype=np.float32) + np.float32(start_pos)
    freqs = t[:, None] * inv[None, :]  # [T, 32]
    emb = np.concatenate([freqs, freqs], axis=-1)  # [T, 64]
    cos = np.cos(emb).T  # [64, T]
    sin = np.sin(emb).T
    sgn = np.where(np.arange(D) % 2 == 0, -1.0, 1.0).astype(np.float32)
    cosT = np.tile(cos, (2, 1))
    sinT = np.tile(sin * sgn[:, None], (2, 1))
    return cosT.astype(NPBF), sinT.astype(NPBF)


def kernel(x, Wq, Wk, Wv, Wp, start_pos):
    x = np.asarray(x, dtype=np.float32)
    Wq = np.asarray(Wq, dtype=np.float32)
    Wk = np.asarray(Wk, dtype=np.float32)
    Wv = np.asarray(Wv, dtype=np.float32)
    Wp = np.asarray(Wp, dtype=np.float32)
    cosT, sinT = _rope_tables(int(start_pos))

    nc = _get_nc()
    in_maps = []
    for c in range(8):
        b, g = divmod(c, 2)
        hs = slice(g * DL, (g + 1) * DL)
        in_maps.append(
            {
                "xT": np.ascontiguousarray(x[b].T).astype(NPBF),
                "wqT": np.ascontiguousarray(Wq[hs, :].T).astype(NPBF),
                "wkT": np.ascontiguousarray(Wk[hs, :].T).astype(NPBF),
                "wvT": np.ascontiguousarray(Wv[hs, :].T).astype(NPBF),
                "wpT": np.ascontiguousarray(Wp[:, hs].T).astype(NPBF),
                "cosT": cosT,
                "sinT": sinT,
            }
        )
    res = run_bass_kernel_spmd(nc, in_maps, core_ids=list(range(8)))
    global LAST_RESULT
    LAST_RESULT = res
    outs = [r["out"] for r in res.results]
    full = np.stack(
        [outs[2 * b] + outs[2 * b + 1] for b in range(B)], axis=0
    )
    return full.astype(np.float32)


if __name__ == "__main__":
    nc = _get_nc()
    print("built ok:", len(nc.m.functions[0].blocks if hasattr(nc.m.functions[0], 'blocks') else []), "blocks")
